# revision 12
# baseline (speedup 1.0000x reference)
"""Trainium2 Bass kernel for nn_Denoiser (GIN-VAE encoder + GAT decoder GNN).

Distribution strategy (8 NeuronCores, SPMD single NEFF):
  - Nodes are sharded by destination ownership: core k owns nodes
    [k*2500, (k+1)*2500). Edges (with self-loops appended) are sorted by dst
    and sharded to the core owning the dst.
  - Edge aggregation (segment-sum / segment-softmax) is computed per 128-dst
    block. Source rows are fetched with ONE batched dma_gather per block
    (cpb*128 indices, 256-B-multiple rows); the one-hot selection matrix for
    the whole block is built with a single broadcast-AP is_equal on the DVE.
  - GIN: per chunk, psum[C, dst] += gt_chunk.T @ sel_chunk (gt stationary).
  - GAT: gathered rows carry [hh0|hh1|a_src pair|pad] (768 B). a_dst pairs
    are fetched from a core-local padded table by block-local dst index.
    exp(leaky(a_s+a_d)) is written into the row tail and the hh halves are
    scaled by it in place; then per chunk psum[dst, 0:258] += sel_chunk.T @
    row[0:258] — both heads' messages AND the softmax denominators in one
    matmul. Division by the denominator is a per-partition scale at PSUM
    evacuation, followed by a PE transpose back to feature-major.
  - Dense per-node math is feature-major ([C partitions, nodes free]) and
    computed own-only; full tables for the next layer's gathers are
    materialized row-major via PE transposes + AllGather (pair-Shared HBM).
  - BatchNorm (training mode, global batch stats) uses a tiny AllReduce of
    per-feature partial sums.

Host-side prep: edge sort/shard/pad, int16 wrapped gather indices, transposed
bf16 inputs, small weight reshapes. All heavy compute runs on device.
"""

import os
import sys

for _p in ("/opt/trn_rl_repo", "/root/.axon_site/_ro/trn_rl_repo"):
    if os.path.isdir(_p) and _p not in sys.path:
        sys.path.insert(0, _p)

from dataclasses import dataclass

import ml_dtypes
import numpy as np

import concourse.bacc as bacc
import concourse.bass as bass
import concourse.mybir as mybir
import concourse.tile as tile
from concourse import library_config
from concourse.bass import AP
from concourse.bass_utils import run_bass_kernel_spmd

F32 = mybir.dt.float32
BF16 = mybir.dt.bfloat16
I16 = mybir.dt.int16
NPBF = ml_dtypes.bfloat16

BN_EPS = 1e-5
SHARED = os.environ.get("KERNEL_SHARED", "1") == "1"


@dataclass
class Cfg:
    n: int = 20000        # total nodes
    ncores: int = 8
    din: int = 92
    c: int = 128          # hidden dim
    h: int = 2            # GAT heads
    cpb: int = 35         # chunks (of 128 edge slots) per dst block
    ts: int = 500         # dense free-dim tile size for own-node matmuls

    @property
    def own(self):
        return self.n // self.ncores

    @property
    def nblk(self):
        return -(-self.own // 128)

    @property
    def slots(self):
        return self.nblk * self.cpb * 128

    @property
    def spb(self):
        return self.cpb * 128  # edge slots per block

    @property
    def ipb(self):
        return self.spb // 16  # idx16 columns per block

    @property
    def bigw(self):
        return 2 * self.c + 128  # [hh0|hh1|a_s0,a_s1,pad] row width (768 B)


def _insert_bcast(ap: AP, pos: int, count: int) -> AP:
    """Insert a [0, count] broadcast dim at position pos of ap's rows."""
    rows = [list(r) for r in ap.ap]
    rows.insert(pos, [0, count])
    return AP(ap.tensor, ap.offset, rows)


def build_program(cfg: Cfg, phase_limit: int = 99) -> bass.Bass:
    nc = bacc.Bacc(
        "TRN2",
        target_bir_lowering=False,
        debug=False,
        enable_asserts=False,
        num_devices=cfg.ncores,
        num_swdge_queues=4,
    )
    n, own, nblk, cpb, ts = cfg.n, cfg.own, cfg.nblk, cfg.cpb, cfg.ts
    C, H, din = cfg.c, cfg.h, cfg.din
    spb, ipb = cfg.spb, cfg.ipb
    bigw = cfg.bigw
    groups = [list(range(cfg.ncores))]
    ntile_own = [min(128, own - t * 128) for t in range(nblk)]  # 128.. tail
    dts = [(i * ts, min(ts, own - i * ts)) for i in range(-(-own // ts))]
    fts = 512
    shared_as = "Shared" if SHARED else "Local"

    # ---------------- I/O ----------------
    di = {}  # dram inputs

    def inp(name, shape, dt):
        di[name] = nc.dram_tensor(name, list(shape), dt, kind="ExternalInput")
        return di[name]

    inp("nfT", [din, own], BF16)            # own node features, transposed
    inp("epsT", [128, own], F32)            # own eps slice, transposed
    inp("w_emb", [din, C], BF16)
    inp("w1", [C, C], BF16)
    inp("w2", [C, C], BF16)
    inp("w_mu", [C, C], BF16)
    inp("w_var", [C, C], BF16)
    inp("w_gat", [C, H * C], BF16)
    inp("w_dec0", [C, C], BF16)             # W_dec rows 0:128
    inp("w_dec1", [C, C], BF16)             # W_dec rows 128:256
    inp("attp0", [C, 2], BF16)              # [att_src[0] | att_dst[0]]
    inp("attp1", [C, 2], BF16)
    inp("bn_emb", [128, 2], F32)            # gamma | beta columns
    inp("bn_gin", [128, 2], F32)
    inp("b2c", [128, 1], F32)
    inp("bmuc", [128, 1], F32)
    inp("bvarc", [128, 1], F32)
    inp("bdecc", [128, 1], F32)             # b_gat @ W_dec + b_dec
    inp("idx_src", [128, nblk * ipb], I16)  # wrapped per-block src ids
    inp("idx_dstl", [128, nblk * ipb], I16)  # wrapped per-block local dst ids
    inp("dstw", [128, cfg.slots // 128], F32)   # block-local dst ids, -1 pad
    inp("iota", [128, 128], BF16)           # row = 0..127 on every partition
    inp("id_bf", [128, 128], BF16)
    inp("id_f32", [128, 128], F32)

    outs = {}
    for nm in ("zin_s", "zout_s", "mu_s", "logvar_s"):
        outs[nm] = nc.dram_tensor(nm, [own, C], F32, kind="ExternalOutput")

    with tile.TileContext(nc) as tc:
        with (
            tc.tile_pool(name="state", bufs=1) as st,
            tc.tile_pool(name="dram", bufs=1, space="DRAM") as dr,
            tc.tile_pool(name="psum_d", bufs=2, space="PSUM") as psd,
            tc.tile_pool(name="psum_t", bufs=2, space="PSUM") as pst,
            tc.tile_pool(name="work", bufs=2) as wk,
        ):
            nc.gpsimd.load_library(library_config.mlp)

            # ---------- load constants / inputs into SBUF ----------
            def load(name, shape, dt, pool=None):
                t = (pool or st).tile(shape, dt, tag=name, name=name)
                nc.sync.dma_start(t[:], di[name][:])
                return t

            w_emb = load("w_emb", [din, C], BF16)
            w1 = load("w1", [C, C], BF16)
            w2 = load("w2", [C, C], BF16)
            w_mu = load("w_mu", [C, C], BF16)
            w_var = load("w_var", [C, C], BF16)
            w_gat = load("w_gat", [C, H * C], BF16)
            w_dec0 = load("w_dec0", [C, C], BF16)
            w_dec1 = load("w_dec1", [C, C], BF16)
            attp = [load("attp0", [C, 2], BF16), load("attp1", [C, 2], BF16)]
            bn_emb = load("bn_emb", [128, 2], F32)
            bn_gin = load("bn_gin", [128, 2], F32)
            b2c = load("b2c", [128, 1], F32)
            bmuc = load("bmuc", [128, 1], F32)
            bvarc = load("bvarc", [128, 1], F32)
            bdecc = load("bdecc", [128, 1], F32)
            idx_src = load("idx_src", [128, nblk * ipb], I16)
            idx_dstl = load("idx_dstl", [128, nblk * ipb], I16)
            dstw = load("dstw", [128, cfg.slots // 128], F32)
            iota = load("iota", [128, 128], BF16)
            id_bf = load("id_bf", [128, 128], BF16)
            id_f32 = load("id_f32", [128, 128], F32)

            # persistent DRAM tables
            x_table = [dr.tile([n, C], BF16, tag=f"x_table{i}",
                               name=f"x_table{i}", addr_space=shared_as)
                       for i in range(2)]
            big_table = [dr.tile([n, bigw], BF16, tag=f"big_table{i}",
                                 name=f"big_table{i}", addr_space=shared_as)
                         for i in range(2)]
            ad_pad = [dr.tile([own, 128], BF16, tag=f"ad_pad{i}",
                              name=f"ad_pad{i}") for i in range(2)]
            cc_rows_big = dr.tile([own, bigw], BF16, tag="cc_rows_big",
                                  name="cc_rows_big")
            cc_rows_x = dr.tile([own, C], BF16, tag="cc_rows_x",
                                name="cc_rows_x")
            cc_stat_in = dr.tile([128, 2], F32, tag="cc_stat_in",
                                 name="cc_stat_in")
            cc_stat_out = [dr.tile([128, 2], F32, tag=f"cc_stat_out{i}",
                                   name=f"cc_stat_out{i}") for i in range(3)]

            # ---------- helpers ----------
            def bn_cols_from_stats(ssum, ssq, gamma_beta, count):
                """Return (k, b) [128,1] f32 columns: y -> y*k + b."""
                mean = wk.tile([128, 1], F32, tag="bn_mean", name="bn_mean")
                nc.vector.tensor_scalar_mul(mean[:], ssum, 1.0 / count)
                ex2 = wk.tile([128, 1], F32, tag="bn_ex2", name="bn_ex2")
                nc.vector.tensor_scalar_mul(ex2[:], ssq, 1.0 / count)
                m2 = wk.tile([128, 1], F32, tag="bn_m2", name="bn_m2")
                nc.vector.tensor_mul(m2[:], mean[:], mean[:])
                var = wk.tile([128, 1], F32, tag="bn_var", name="bn_var")
                nc.vector.tensor_sub(var[:], ex2[:], m2[:])
                nc.vector.tensor_scalar_add(var[:], var[:], BN_EPS)
                inv = wk.tile([128, 1], F32, tag="bn_inv", name="bn_inv")
                nc.vector.reciprocal(inv[:], var[:])
                rs = wk.tile([128, 1], F32, tag="bn_rs", name="bn_rs")
                nc.scalar.sqrt(rs[:], inv[:])
                k = wk.tile([128, 1], F32, tag="bn_k", name="bn_k")
                nc.vector.tensor_mul(k[:], rs[:], gamma_beta[:, 0:1])
                mk = wk.tile([128, 1], F32, tag="bn_mk", name="bn_mk")
                nc.vector.tensor_mul(mk[:], mean[:], k[:])
                b = wk.tile([128, 1], F32, tag="bn_b", name="bn_b")
                nc.vector.tensor_sub(b[:], gamma_beta[:, 1:2], mk[:])
                return k, b

            def stats_of(ytile, width):
                """Local per-feature sum and sum-of-squares of y [128,width]."""
                ssum = wk.tile([128, 1], F32, tag="st_ssum", name="st_ssum")
                nc.vector.tensor_reduce(
                    ssum[:], ytile[:, 0:width], axis=mybir.AxisListType.X,
                    op=mybir.AluOpType.add,
                )
                sq = wk.tile([128, len(dts)], F32, tag="st_sq", name="st_sq")
                for i, (o0, w_) in enumerate(dts):
                    scr = wk.tile([128, fts], BF16, tag="scr0", name="scr0")
                    nc.scalar.activation(
                        scr[:, 0:w_], ytile[:, o0 : o0 + w_],
                        mybir.ActivationFunctionType.Square,
                        accum_out=sq[:, i : i + 1],
                    )
                ssq = wk.tile([128, 1], F32, tag="st_ssq", name="st_ssq")
                nc.vector.tensor_reduce(
                    ssq[:], sq[:], axis=mybir.AxisListType.X,
                    op=mybir.AluOpType.add
                )
                return ssum, ssq

            def allreduce_stats(ssum, ssq, idx):
                statloc = wk.tile([128, 2], F32, tag="statloc", name="statloc")
                nc.vector.tensor_copy(statloc[:, 0:1], ssum[:])
                nc.vector.tensor_copy(statloc[:, 1:2], ssq[:])
                nc.sync.dma_start(cc_stat_in[:, :], statloc[:])
                nc.gpsimd.collective_compute(
                    "AllReduce", mybir.AluOpType.add, groups,
                    [cc_stat_in[:, :].opt()], [cc_stat_out[idx][:, :].opt()],
                )
                statglob = wk.tile([128, 2], F32, tag="statglob",
                                   name="statglob")
                nc.sync.dma_start(statglob[:], cc_stat_out[idx][:, :])
                return statglob

            GW = 8  # node tiles per batched row-write DMA

            def write_rows(cols, dst_dram, width, src_dt, grp=GW,
                           fill_pad=False):
                """Transpose f-major own tiles into row-major dst_dram.

                cols: list of (col_off, ncol, get_ap(t, nt) -> [ncol, nt] AP).
                """
                full = own // 128
                tail = own % 128
                ident_t = id_bf if src_dt == BF16 else id_f32

                def rowbuf_tile():
                    rb = wk.tile([128, grp * width], src_dt,
                                 tag=f"rows{width}_{src_dt}",
                                 name=f"rows{width}_{src_dt}")
                    if fill_pad:
                        nc.vector.memset(rb[:], 0.0)
                    return rb

                def do_tile(rowbuf, t, nt, g):
                    for (co, ncol, get) in cols:
                        src = get(t, nt)
                        pt = pst.tile([128, 128], src_dt, tag="t", name="pt")
                        nc.tensor.transpose(
                            pt[0:nt, 0:ncol], src, ident_t[0:ncol, 0:ncol]
                        )
                        nc.scalar.copy(
                            rowbuf[0:nt, g * width + co : g * width + co + ncol],
                            pt[0:nt, 0:ncol],
                        )

                t = 0
                while t < full:
                    gcnt = min(grp, full - t)
                    rowbuf = rowbuf_tile()
                    for g in range(gcnt):
                        do_tile(rowbuf, t + g, 128, g)
                    if gcnt == 1:
                        nc.sync.dma_start(
                            dst_dram[t * 128 : (t + 1) * 128, :],
                            rowbuf[:, 0:width],
                        )
                    else:
                        dst = dst_dram[t * 128 : (t + gcnt) * 128, :].rearrange(
                            "(g p) f -> p g f", p=128
                        )
                        nc.sync.dma_start(
                            dst,
                            rowbuf[:, 0 : gcnt * width].rearrange(
                                "p (g f) -> p g f", f=width
                            ),
                        )
                    t += gcnt
                if tail:
                    rowbuf = rowbuf_tile()
                    do_tile(rowbuf, full, tail, 0)
                    nc.sync.dma_start(
                        dst_dram[full * 128 : own, :], rowbuf[0:tail, 0:width]
                    )

            def dense_own(lhsTs, rhs_fn, out_tag, out_dt, pool, bias=None,
                          act=None, scale=None):
                """out[128, own] = act(scale * sum_i lhsTs[i].T @ rhs_i + bias)."""
                o = pool.tile([128, own], out_dt, tag=out_tag, name=out_tag)
                if act is not None:
                    func = act
                elif bias is None and scale is None:
                    func = mybir.ActivationFunctionType.Copy
                else:
                    func = mybir.ActivationFunctionType.Identity
                for (o0, w_) in dts:
                    pt = psd.tile([128, 512], F32, tag="d", name="pd")
                    for i, (lt, rf) in enumerate(zip(lhsTs, rhs_fn)):
                        nc.tensor.matmul(
                            pt[:, 0:w_], lt, rf(o0, w_),
                            start=(i == 0), stop=(i == len(lhsTs) - 1),
                        )
                    kw = {}
                    if bias is not None:
                        kw["bias"] = bias
                    if scale is not None:
                        kw["scale"] = scale
                    nc.scalar.activation(
                        o[:, o0 : o0 + w_], pt[:, 0:w_], func, **kw,
                    )
                return o

            qrr = [0]  # SWDGE queue round-robin: spread desc-gen over Q7 pairs
            cpb2 = cpb // 2

            def gather_block(pool, table, idx16, b, width, tag, bufs=3):
                """Gather one block's spb rows into [128,cpb,width], split into
                two half-block dma_gathers on different SWDGE queues so the
                Q7 descriptor generation runs on two core pairs in parallel."""
                t = pool.tile([128, cpb, width], BF16, tag=tag, name=tag,
                              bufs=bufs)
                for (c0, c1) in ((0, cpb2), (cpb2, cpb)):
                    nchunk = c1 - c0
                    nc.gpsimd.dma_gather(
                        t[:, c0:c1, :], table[:, :],
                        idx16[:, b * ipb + c0 * 8 : b * ipb + c1 * 8],
                        nchunk * 128, nchunk * 128, width,
                        single_packet=False, queue_num=qrr[0],
                    )
                    qrr[0] = (qrr[0] + 1) % 4
                return t

            def sel_block(pool, b, tag, bufs=2):
                """sel[e, c, d] = (iota_d == dstw[e, c]) for one block.

                Built per chunk with tensor_scalar (per-partition scalar =
                dstw column) — eligible for the DVE 4x bf16 perf mode, unlike
                a broadcast-AP tensor_tensor which drops to 1x.
                """
                s = pool.tile([128, cpb, 128], BF16, tag=tag, name=tag,
                              bufs=bufs)
                for c in range(cpb):
                    col = b * cpb + c
                    nc.vector.tensor_scalar(
                        s[:, c, :], iota[:], dstw[:, col : col + 1], None,
                        mybir.AluOpType.is_equal,
                    )
                return s

            # =================================================================
            # Phase 0: embedding (own slice) + stats AllReduce + x0 AllGather
            # =================================================================
            with tc.tile_pool(name="p0", bufs=1) as p0:
                nfT = load("nfT", [din, own], BF16, pool=p0)
                y0 = p0.tile([128, own], BF16, tag="y0", name="y0")
                for (o0, w_) in dts:
                    pt = psd.tile([128, 512], F32, tag="d", name="pd")
                    nc.tensor.matmul(
                        pt[:, 0:w_], w_emb[:], nfT[:, o0 : o0 + w_],
                        start=True, stop=True,
                    )
                    nc.scalar.copy(y0[:, o0 : o0 + w_], pt[:, 0:w_])
                ssum0, ssq0 = stats_of(y0, own)
                statglob = allreduce_stats(ssum0, ssq0, 2)
                k0, b0 = bn_cols_from_stats(
                    statglob[:, 0:1], statglob[:, 1:2], bn_emb, n
                )
                x0 = p0.tile([128, own], BF16, tag="x0", name="x0")
                nc.scalar.activation(
                    x0[:], y0[:], mybir.ActivationFunctionType.Relu,
                    bias=b0[:], scale=k0[:],
                )
                write_rows(
                    [(0, C, lambda t, nt: x0[:, t * 128 : t * 128 + nt])],
                    cc_rows_x, C, BF16,
                )
                nc.gpsimd.collective_compute(
                    "AllGather", mybir.AluOpType.bypass, groups,
                    [cc_rows_x[:, :].opt()], [x_table[0][:, :].opt()],
                )

            # =================================================================
            # GIN layers (x2): edge aggregation + own-only dense MLP
            # =================================================================
            def gin_layer(li, sp, out_pool):
                h = sp.tile([128, own], BF16, tag=f"h_gin{li}",
                            name=f"h_gin{li}")
                with (
                    tc.tile_pool(name=f"gin_e{li}", bufs=2) as ep,
                    tc.tile_pool(name=f"gin_p{li}", bufs=2, space="PSUM") as pp,
                ):
                    for b in range(nblk):
                        gt = gather_block(ep, x_table[li], idx_src, b, C, "gt")
                        s = sel_block(ep, b, "sel")
                        pa = pp.tile([128, 128], F32, tag="agg", name="agg")
                        for c in range(cpb):
                            nc.tensor.matmul(
                                pa[:], gt[:, c, :], s[:, c, :],
                                start=(c == 0), stop=(c == cpb - 1),
                            )
                        nb = ntile_own[b]
                        nc.scalar.copy(
                            h[:, b * 128 : b * 128 + nb], pa[:, 0:nb]
                        )
                # dense: y = h @ W1; BN(global) + relu; x = h2 @ W2 + b2
                y = dense_own([w1[:]], [lambda o0, w_: h[:, o0 : o0 + w_]],
                              f"y_gin{li}", BF16, sp)
                ssum, ssq = stats_of(y, own)
                statglob = allreduce_stats(ssum, ssq, li)
                k, bcol = bn_cols_from_stats(
                    statglob[:, 0:1], statglob[:, 1:2], bn_gin, n
                )
                h2 = sp.tile([128, own], BF16, tag=f"h2_gin{li}",
                             name=f"h2_gin{li}")
                nc.scalar.activation(
                    h2[:], y[:], mybir.ActivationFunctionType.Relu,
                    bias=bcol[:], scale=k[:],
                )
                x = dense_own(
                    [w2[:]], [lambda o0, w_: h2[:, o0 : o0 + w_]],
                    f"x_gin{li}", BF16, out_pool,
                    bias=b2c[:], act=mybir.ActivationFunctionType.Identity,
                )
                if li == 0:
                    write_rows(
                        [(0, C, lambda t, nt: x[:, t * 128 : t * 128 + nt])],
                        cc_rows_x, C, BF16,
                    )
                    nc.gpsimd.collective_compute(
                        "AllGather", mybir.AluOpType.bypass, groups,
                        [cc_rows_x[:, :].opt()], [x_table[1][:, :].opt()],
                    )
                return x

            if phase_limit >= 1:
                with tc.tile_pool(name="g0", bufs=1) as g0p:
                    gin_layer(0, g0p, g0p)

            zb = st.tile([128, own], BF16, tag="zb", name="zb")
            with tc.tile_pool(name="g1", bufs=1) as g1p:
                if phase_limit < 2:
                    nc.vector.memset(zb[:], 0.0)
                    x2 = None
                else:
                    x2 = gin_layer(1, g1p, g1p)

                # =============================================================
                # VAE heads (own only)
                # =============================================================
                if phase_limit < 3:
                    zt = wk.tile([128, 512], F32, tag="zf", name="zf")
                    nc.vector.memset(zt[:], 0.0)
                    for nm in ("mu_s", "logvar_s", "zin_s"):
                        for t in range(nblk):
                            nt = ntile_own[t]
                            nc.sync.dma_start(
                                outs[nm][t * 128 : t * 128 + nt, :],
                                zt[0:nt, 0:C])
                    mu = None
                else:
                    mu = dense_own(
                        [w_mu[:]], [lambda o0, w_: x2[:, o0 : o0 + w_]],
                        "mu", F32, g1p, bias=bmuc[:],
                        act=mybir.ActivationFunctionType.Identity)
                if phase_limit >= 3:
                    lv = dense_own(
                        [w_var[:]], [lambda o0, w_: x2[:, o0 : o0 + w_]],
                        "lv", F32, g1p, bias=bvarc[:],
                        act=mybir.ActivationFunctionType.Identity)
                    eh = g1p.tile([128, own], F32, tag="eh", name="eh")
                    nc.scalar.activation(
                        eh[:], lv[:], mybir.ActivationFunctionType.Exp,
                        scale=0.5
                    )
                    epsT = load("epsT", [128, own], F32, pool=g1p)
                    z = g1p.tile([128, own], F32, tag="z", name="z")
                    nc.vector.tensor_mul(z[:], epsT[:], eh[:])
                    nc.vector.tensor_add(z[:], z[:], mu[:])
                    nc.vector.tensor_copy(zb[:], z[:])
                    write_rows(
                        [(0, C, lambda t, nt: mu[:, t * 128 : t * 128 + nt])],
                        outs["mu_s"], C, F32)
                    write_rows(
                        [(0, C, lambda t, nt: lv[:, t * 128 : t * 128 + nt])],
                        outs["logvar_s"], C, F32)
                    write_rows(
                        [(0, C, lambda t, nt: z[:, t * 128 : t * 128 + nt])],
                        outs["zin_s"], C, F32)

            # =================================================================
            # GAT layers (x2)
            # =================================================================
            def gat_tables(li, act):
                """act [128, own] bf16 -> big_table[li] (AllGather), ad_pad[li]
                (local)."""
                with tc.tile_pool(name=f"gtab{li}", bufs=1) as tp:
                    hh = [
                        dense_own(
                            [w_gat[:, hd * C : (hd + 1) * C]],
                            [lambda o0, w_: act[:, o0 : o0 + w_]],
                            f"hh{hd}_l{li}", BF16, tp,
                        )
                        for hd in range(H)
                    ]
                    # a[kind][hd]: [1, own] row, kind 0 = a_src, 1 = a_dst
                    arow = [[None, None], [None, None]]
                    for hd in range(H):
                        for kind in range(2):
                            t_ = tp.tile([1, own], BF16,
                                         tag=f"a{kind}{hd}_l{li}",
                                         name=f"a{kind}{hd}_l{li}")
                            for (o0, w_) in dts:
                                pt = psd.tile([1, 512], F32, tag="d", name="pd")
                                nc.tensor.matmul(
                                    pt[0:1, 0:w_],
                                    attp[hd][:, kind : kind + 1],
                                    hh[hd][:, o0 : o0 + w_],
                                    start=True, stop=True,
                                )
                                nc.scalar.copy(t_[0:1, o0 : o0 + w_],
                                               pt[0:1, 0:w_])
                            arow[kind][hd] = t_
                    write_rows(
                        [
                            (0, C,
                             lambda t, nt: hh[0][:, t * 128 : t * 128 + nt]),
                            (C, C,
                             lambda t, nt: hh[1][:, t * 128 : t * 128 + nt]),
                            (2 * C, 1,
                             lambda t, nt: arow[0][0][0:1,
                                                      t * 128 : t * 128 + nt]),
                            (2 * C + 1, 1,
                             lambda t, nt: arow[0][1][0:1,
                                                      t * 128 : t * 128 + nt]),
                        ],
                        cc_rows_big, bigw, BF16, fill_pad=True,
                    )
                    write_rows(
                        [
                            (0, 1,
                             lambda t, nt: arow[1][0][0:1,
                                                      t * 128 : t * 128 + nt]),
                            (1, 1,
                             lambda t, nt: arow[1][1][0:1,
                                                      t * 128 : t * 128 + nt]),
                        ],
                        ad_pad[li], 128, BF16, fill_pad=True,
                    )
                nc.gpsimd.collective_compute(
                    "AllGather", mybir.AluOpType.bypass, groups,
                    [cc_rows_big[:, :].opt()], [big_table[li][:, :].opt()],
                )

            def gat_edge_dec(li, out_pool, out_dt):
                """Edge softmax + message aggregation + decoder matmul.

                Per chunk: psum[dst, 0:258] += sel.T @ [el0*hh0|el1*hh1|el0,el1]
                so the denominator comes out of the same matmul; division is a
                per-partition scale at evacuation, then PE transpose to
                feature-major.
                """
                with (
                    tc.tile_pool(name=f"gat_s{li}", bufs=1) as gsp,
                    tc.tile_pool(name=f"gat_e{li}", bufs=2) as ep,
                    tc.tile_pool(name=f"gat_p{li}", bufs=2, space="PSUM") as pp,
                ):
                    on = [gsp.tile([128, own], BF16, tag=f"on{hd}",
                                   name=f"on{hd}")
                          for hd in range(H)]
                    for b in range(nblk):
                        gt = gather_block(ep, big_table[li], idx_src, b, bigw,
                                          "gt")
                        adg = gather_block(ep, ad_pad[li], idx_dstl, b, 128,
                                           "adg", bufs=2)
                        s = sel_block(ep, b, "sel")
                        # el = exp(leaky(a_s + a_d)) -> gt[:, :, 256:258]
                        lt = ep.tile([128, cpb, 2], F32, tag="lt", name="lt")
                        nc.vector.tensor_add(
                            lt[:, :, :], gt[:, :, 2 * C : 2 * C + 2],
                            adg[:, :, 0:2],
                        )
                        nc.vector.scalar_tensor_tensor(
                            lt[:, :, :], lt[:, :, :], 0.2, lt[:, :, :],
                            mybir.AluOpType.mult, mybir.AluOpType.max,
                        )
                        elf = ep.tile([128, cpb, 2], F32, tag="elf",
                                      name="elf")
                        nc.scalar.activation(
                            elf[:, :, :], lt[:, :, :],
                            mybir.ActivationFunctionType.Exp,
                        )
                        nc.vector.tensor_copy(gt[:, :, 2 * C : 2 * C + 2],
                                              elf[:, :, :])
                        # scale hh halves by el in place (tensor_scalar: 4x)
                        for c in range(cpb):
                            for hd in range(H):
                                nc.vector.tensor_scalar_mul(
                                    gt[:, c, hd * C : (hd + 1) * C],
                                    gt[:, c, hd * C : (hd + 1) * C],
                                    elf[:, c, hd : hd + 1],
                                )
                        pa = pp.tile([128, 258], F32, tag="pa", name="pa")
                        for c in range(cpb):
                            nc.tensor.matmul(
                                pa[:, 0:258], s[:, c, :],
                                gt[:, c, 0 : 2 * C + 2],
                                start=(c == 0), stop=(c == cpb - 1),
                            )
                        nb = ntile_own[b]
                        r2 = wk.tile([128, 2], F32, tag="r2", name="r2")
                        nc.vector.reciprocal(r2[0:nb, :],
                                             pa[0:nb, 2 * C : 2 * C + 2])
                        for hd in range(H):
                            sb = wk.tile([128, 128], BF16, tag="sb", name="sb")
                            nc.scalar.activation(
                                sb[0:nb, :],
                                pa[0:nb, hd * C : (hd + 1) * C],
                                mybir.ActivationFunctionType.Identity,
                                scale=r2[0:nb, hd : hd + 1],
                            )
                            ptr = pst.tile([128, 128], BF16, tag="t",
                                           name="ptr")
                            nc.tensor.transpose(
                                ptr[0:128, 0:nb], sb[0:nb, 0:128],
                                id_bf[0:nb, 0:nb],
                            )
                            nc.scalar.copy(
                                on[hd][:, b * 128 : b * 128 + nb],
                                ptr[:, 0:nb],
                            )
                    # decoder: zout = (out + b_gat) @ W_dec + b_dec (prefolded)
                    zo = dense_own(
                        [w_dec0[:], w_dec1[:]],
                        [lambda o0, w_: on[0][:, o0 : o0 + w_],
                         lambda o0, w_: on[1][:, o0 : o0 + w_]],
                        f"zo_l{li}", out_dt, out_pool,
                        bias=bdecc[:], act=mybir.ActivationFunctionType.Identity,
                    )
                return zo

            if phase_limit >= 4:
                gat_tables(0, zb)
            if phase_limit >= 5:
                zo1 = gat_edge_dec(0, st, BF16)
            if phase_limit >= 6:
                gat_tables(1, zo1)
            if phase_limit >= 7:
                with tc.tile_pool(name="zo2p", bufs=1) as zp:
                    zo2 = gat_edge_dec(1, zp, F32)
                    write_rows(
                        [(0, C, lambda t, nt: zo2[:, t * 128 : t * 128 + nt])],
                        outs["zout_s"], C, F32,
                    )
            else:
                zt0 = wk.tile([128, 512], F32, tag="zf", name="zf")
                nc.vector.memset(zt0[:], 0.0)
                for t in range(nblk):
                    nt = ntile_own[t]
                    nc.sync.dma_start(
                        outs["zout_s"][t * 128 : t * 128 + nt, :],
                        zt0[0:nt, 0:C])
    nc.compile()
    return nc


# =====================================================================
# Host side
# =====================================================================
def host_prep(edge_index, cfg: Cfg):
    n, ncores, own, nblk = cfg.n, cfg.ncores, cfg.own, cfg.nblk
    src = np.asarray(edge_index[0], dtype=np.int64)
    dst = np.asarray(edge_index[1], dtype=np.int64)
    loop = np.arange(n, dtype=np.int64)
    src = np.concatenate([src, loop])
    dst = np.concatenate([dst, loop])
    order = np.argsort(dst, kind="stable")
    src, dst = src[order], dst[order]
    # bucket edges by (core, block)
    core_of = dst // own
    blk_in_core = (dst - core_of * own) // 128
    counts = np.zeros((ncores, nblk), dtype=np.int64)
    np.add.at(counts, (core_of, blk_in_core), 1)
    cpb = int(-(-counts.max() // 128))
    cfg.cpb = cpb
    slots = cfg.slots
    idx_src = np.zeros((ncores, slots), dtype=np.int64)
    idx_dstl = np.zeros((ncores, slots), dtype=np.int64)
    dstl = np.full((ncores, slots), -1.0, dtype=np.float32)
    # edges are sorted by dst, so per (core, block) they are contiguous
    for c in range(ncores):
        for b in range(nblk):
            b_lo = np.searchsorted(dst, c * own + b * 128)
            b_hi = np.searchsorted(dst,
                                   min(c * own + (b + 1) * 128, (c + 1) * own))
            cnt = b_hi - b_lo
            s0 = b * cpb * 128
            idx_src[c, s0 : s0 + cnt] = src[b_lo:b_hi]
            idx_dstl[c, s0 : s0 + cnt] = dst[b_lo:b_hi] - c * own
            dstl[c, s0 : s0 + cnt] = (
                dst[b_lo:b_hi] - c * own - b * 128
            ).astype(np.float32)

    def wrap16(a):
        # slot j of each block -> [j%16 + 16k, j//16] within the block cols
        blocks = a.reshape(nblk, cfg.spb)
        w = np.stack([np.tile(blk.reshape(cfg.ipb, 16).T, (8, 1))
                      for blk in blocks], axis=0)  # [nblk, 128, ipb]
        return np.ascontiguousarray(
            w.transpose(1, 0, 2).reshape(128, nblk * cfg.ipb)
        ).astype(np.int16)

    per_core = {}
    for c in range(ncores):
        per_core[c] = dict(
            idx_src=wrap16(idx_src[c]),
            idx_dstl=wrap16(idx_dstl[c]),
            dstw=np.ascontiguousarray(
                dstl[c].reshape(slots // 128, 128).T
            ),
        )
    return per_core


def kernel(node_features_s, edge_index_s, eps_noise,
           W_emb, b_emb, g_emb, be_emb,
           W1, b1, g1, be1, W2, b2,
           W_mu, b_mu, W_var, b_var,
           W_gat, att_src, att_dst, b_gat,
           W_dec, b_dec, _cfg=None, _nc_cache={}):
    cfg = _cfg or Cfg()
    n, own, C, H = cfg.n, cfg.own, cfg.c, cfg.h
    per_core = host_prep(edge_index_s, cfg)

    nf = np.asarray(node_features_s, dtype=np.float32)
    eps = np.asarray(eps_noise, dtype=np.float32)

    def colpair(a, b_):
        return np.stack([np.asarray(a, np.float32).reshape(-1),
                         np.asarray(b_, np.float32).reshape(-1)], axis=1)

    iota = np.tile(np.arange(128, dtype=np.float32), (128, 1))
    bdec_eff = (np.asarray(b_gat, np.float32) @ np.asarray(W_dec, np.float32)
                + np.asarray(b_dec, np.float32))
    shared = dict(
        w_emb=np.asarray(W_emb, np.float32).astype(NPBF),
        w1=np.asarray(W1, np.float32).astype(NPBF),
        w2=np.asarray(W2, np.float32).astype(NPBF),
        w_mu=np.asarray(W_mu, np.float32).astype(NPBF),
        w_var=np.asarray(W_var, np.float32).astype(NPBF),
        w_gat=np.asarray(W_gat, np.float32).astype(NPBF),
        w_dec0=np.asarray(W_dec, np.float32)[:C].astype(NPBF),
        w_dec1=np.asarray(W_dec, np.float32)[C:].astype(NPBF),
        attp0=np.stack([np.asarray(att_src, np.float32)[0],
                        np.asarray(att_dst, np.float32)[0]],
                       axis=1).astype(NPBF),
        attp1=np.stack([np.asarray(att_src, np.float32)[1],
                        np.asarray(att_dst, np.float32)[1]],
                       axis=1).astype(NPBF),
        bn_emb=colpair(g_emb, be_emb),
        bn_gin=colpair(g1, be1),
        b2c=np.asarray(b2, np.float32).reshape(C, 1),
        bmuc=np.asarray(b_mu, np.float32).reshape(C, 1),
        bvarc=np.asarray(b_var, np.float32).reshape(C, 1),
        bdecc=bdec_eff.reshape(C, 1),
        iota=iota.astype(NPBF),
        id_bf=np.eye(128, dtype=np.float32).astype(NPBF),
        id_f32=np.eye(128, dtype=np.float32),
    )
    in_maps = []
    for c in range(cfg.ncores):
        m = dict(shared)
        m["nfT"] = np.ascontiguousarray(
            nf[c * own : (c + 1) * own].T).astype(NPBF)
        m["epsT"] = np.ascontiguousarray(eps[c * own : (c + 1) * own].T)
        m.update(per_core[c])
        in_maps.append(m)

    pl = int(os.environ.get("KERNEL_PHASES", "99"))
    key = (cfg.n, cfg.ncores, cfg.cpb, pl, SHARED)
    if key not in _nc_cache:
        _nc_cache[key] = build_program(cfg, phase_limit=pl)
    nc = _nc_cache[key]

    res = run_bass_kernel_spmd(
        nc, in_maps, core_ids=list(range(cfg.ncores)),
        trace=bool(int(os.environ.get("KERNEL_TRACE", "0"))),
    )
    results = res.results
    kernel.last_run = res

    def stitch(name):
        return np.concatenate([np.asarray(results[c][name], np.float32)
                               for c in range(cfg.ncores)], axis=0)

    return (stitch("zin_s"), stitch("zout_s"), stitch("mu_s"),
            stitch("logvar_s"))


# revision 14
# speedup vs baseline: 1.4454x; 1.4454x over previous
"""Trainium2 Bass kernel for nn_Denoiser (GIN-VAE encoder + GAT decoder GNN).

Distribution strategy (8 NeuronCores, SPMD single NEFF):
  - Nodes are sharded by destination ownership: core k owns nodes
    [k*2500, (k+1)*2500). Edges (with self-loops appended) are sorted by dst
    and sharded to the core owning the dst.
  - Edge aggregation (segment-sum / segment-softmax) is computed per 128-dst
    block. Source rows are fetched with ONE batched dma_gather per block
    (cpb*128 indices, 256-B-multiple rows); the one-hot selection matrix for
    the whole block is built with a single broadcast-AP is_equal on the DVE.
  - GIN: per chunk, psum[C, dst] += gt_chunk.T @ sel_chunk (gt stationary).
  - GAT: gathered rows carry [hh0|hh1|a_src pair|pad] (768 B). a_dst pairs
    are fetched from a core-local padded table by block-local dst index.
    exp(leaky(a_s+a_d)) is written into the row tail and the hh halves are
    scaled by it in place; then per chunk psum[dst, 0:258] += sel_chunk.T @
    row[0:258] — both heads' messages AND the softmax denominators in one
    matmul. Division by the denominator is a per-partition scale at PSUM
    evacuation, followed by a PE transpose back to feature-major.
  - Dense per-node math is feature-major ([C partitions, nodes free]) and
    computed own-only; full tables for the next layer's gathers are
    materialized row-major via PE transposes + AllGather (pair-Shared HBM).
  - BatchNorm (training mode, global batch stats) uses a tiny AllReduce of
    per-feature partial sums.

Host-side prep: edge sort/shard/pad, int16 wrapped gather indices, transposed
bf16 inputs, small weight reshapes. All heavy compute runs on device.
"""

import os
import sys

for _p in ("/opt/trn_rl_repo", "/root/.axon_site/_ro/trn_rl_repo"):
    if os.path.isdir(_p) and _p not in sys.path:
        sys.path.insert(0, _p)

from dataclasses import dataclass

import ml_dtypes
import numpy as np

import concourse.bacc as bacc
import concourse.bass as bass
import concourse.mybir as mybir
import concourse.tile as tile
from concourse import library_config
from concourse.bass import AP
from concourse.bass_utils import run_bass_kernel_spmd

F32 = mybir.dt.float32
BF16 = mybir.dt.bfloat16
I16 = mybir.dt.int16
NPBF = ml_dtypes.bfloat16

BN_EPS = 1e-5
SHARED = os.environ.get("KERNEL_SHARED", "1") == "1"


@dataclass
class Cfg:
    n: int = 20000        # total nodes
    ncores: int = 8
    din: int = 92
    c: int = 128          # hidden dim
    h: int = 2            # GAT heads
    cpb: int = 35         # chunks (of 128 edge slots) per dst block
    ts: int = 500         # dense free-dim tile size for own-node matmuls

    @property
    def own(self):
        return self.n // self.ncores

    @property
    def nblk(self):
        return -(-self.own // 128)

    @property
    def slots(self):
        return self.nblk * self.cpb * 128

    @property
    def spb(self):
        return self.cpb * 128  # edge slots per block

    @property
    def ipb(self):
        return self.spb // 16  # idx16 columns per block

    @property
    def bigw(self):
        return 2 * self.c + 128  # [hh0|hh1|a_s0,a_s1,pad] row width (768 B)


def _insert_bcast(ap: AP, pos: int, count: int) -> AP:
    """Insert a [0, count] broadcast dim at position pos of ap's rows."""
    rows = [list(r) for r in ap.ap]
    rows.insert(pos, [0, count])
    return AP(ap.tensor, ap.offset, rows)


def build_program(cfg: Cfg, phase_limit: int = 99) -> bass.Bass:
    nc = bacc.Bacc(
        "TRN2",
        target_bir_lowering=False,
        debug=False,
        enable_asserts=False,
        num_devices=cfg.ncores,
        num_swdge_queues=4,
    )
    n, own, nblk, cpb, ts = cfg.n, cfg.own, cfg.nblk, cfg.cpb, cfg.ts
    C, H, din = cfg.c, cfg.h, cfg.din
    spb, ipb = cfg.spb, cfg.ipb
    bigw = cfg.bigw
    groups = [list(range(cfg.ncores))]
    ntile_own = [min(128, own - t * 128) for t in range(nblk)]  # 128.. tail
    dts = [(i * ts, min(ts, own - i * ts)) for i in range(-(-own // ts))]
    fts = 512
    shared_as = "Shared" if SHARED else "Local"

    # ---------------- I/O ----------------
    di = {}  # dram inputs

    def inp(name, shape, dt):
        di[name] = nc.dram_tensor(name, list(shape), dt, kind="ExternalInput")
        return di[name]

    inp("nfT", [din, own], BF16)            # own node features, transposed
    inp("epsT", [128, own], F32)            # own eps slice, transposed
    inp("w_emb", [din, C], BF16)
    inp("w1", [C, C], BF16)
    inp("w2", [C, C], BF16)
    inp("w_mu", [C, C], BF16)
    inp("w_var", [C, C], BF16)
    inp("w_gat", [C, H * C], BF16)
    inp("w_dec0", [C, C], BF16)             # W_dec rows 0:128
    inp("w_dec1", [C, C], BF16)             # W_dec rows 128:256
    inp("attp0", [C, 2], BF16)              # [att_src[0] | att_dst[0]]
    inp("attp1", [C, 2], BF16)
    inp("bn_emb", [128, 2], F32)            # gamma | beta columns
    inp("bn_gin", [128, 2], F32)
    inp("b2c", [128, 1], F32)
    inp("bmuc", [128, 1], F32)
    inp("bvarc", [128, 1], F32)
    inp("bdecc", [128, 1], F32)             # b_gat @ W_dec + b_dec
    inp("idx_src", [128, nblk * ipb], I16)  # wrapped per-block src ids
    inp("idx_dstl", [128, nblk * ipb], I16)  # wrapped per-block local dst ids
    inp("dstw", [128, cfg.slots // 128], F32)   # block-local dst ids, -1 pad
    inp("iota", [128, 128], BF16)           # row = 0..127 on every partition
    inp("id_bf", [128, 128], BF16)
    inp("id_f32", [128, 128], F32)

    outs = {}
    for nm in ("zin_s", "zout_s", "mu_s", "logvar_s"):
        outs[nm] = nc.dram_tensor(nm, [own, C], F32, kind="ExternalOutput")

    with tile.TileContext(nc) as tc:
        with (
            tc.tile_pool(name="state", bufs=1) as st,
            tc.tile_pool(name="dram", bufs=1, space="DRAM") as dr,
            tc.tile_pool(name="psum_d", bufs=2, space="PSUM") as psd,
            tc.tile_pool(name="psum_t", bufs=2, space="PSUM") as pst,
            tc.tile_pool(name="work", bufs=2) as wk,
        ):
            nc.gpsimd.load_library(library_config.mlp)

            # ---------- load constants / inputs into SBUF ----------
            def load(name, shape, dt, pool=None):
                t = (pool or st).tile(shape, dt, tag=name, name=name)
                nc.sync.dma_start(t[:], di[name][:])
                return t

            w_emb = load("w_emb", [din, C], BF16)
            w1 = load("w1", [C, C], BF16)
            w2 = load("w2", [C, C], BF16)
            w_mu = load("w_mu", [C, C], BF16)
            w_var = load("w_var", [C, C], BF16)
            w_gat = load("w_gat", [C, H * C], BF16)
            w_dec0 = load("w_dec0", [C, C], BF16)
            w_dec1 = load("w_dec1", [C, C], BF16)
            attp = [load("attp0", [C, 2], BF16), load("attp1", [C, 2], BF16)]
            bn_emb = load("bn_emb", [128, 2], F32)
            bn_gin = load("bn_gin", [128, 2], F32)
            b2c = load("b2c", [128, 1], F32)
            bmuc = load("bmuc", [128, 1], F32)
            bvarc = load("bvarc", [128, 1], F32)
            bdecc = load("bdecc", [128, 1], F32)
            idx_src = load("idx_src", [128, nblk * ipb], I16)
            idx_dstl = load("idx_dstl", [128, nblk * ipb], I16)
            dstw = load("dstw", [128, cfg.slots // 128], F32)
            iota = load("iota", [128, 128], BF16)
            id_bf = load("id_bf", [128, 128], BF16)
            id_f32 = load("id_f32", [128, 128], F32)

            # persistent DRAM tables
            x_table = [dr.tile([n, C], BF16, tag=f"x_table{i}",
                               name=f"x_table{i}", addr_space=shared_as)
                       for i in range(2)]
            big_table = [dr.tile([n, bigw], BF16, tag=f"big_table{i}",
                                 name=f"big_table{i}", addr_space=shared_as)
                         for i in range(2)]
            ad_pad = [dr.tile([own, 128], BF16, tag=f"ad_pad{i}",
                              name=f"ad_pad{i}") for i in range(2)]
            cc_rows_big = dr.tile([own, bigw], BF16, tag="cc_rows_big",
                                  name="cc_rows_big")
            cc_rows_x = dr.tile([own, C], BF16, tag="cc_rows_x",
                                name="cc_rows_x")
            cc_stat_in = dr.tile([128, 2], F32, tag="cc_stat_in",
                                 name="cc_stat_in")
            cc_stat_out = [dr.tile([128, 2], F32, tag=f"cc_stat_out{i}",
                                   name=f"cc_stat_out{i}") for i in range(3)]

            # ---------- helpers ----------
            def bn_cols_from_stats(ssum, ssq, gamma_beta, count):
                """Return (k, b) [128,1] f32 columns: y -> y*k + b."""
                mean = wk.tile([128, 1], F32, tag="bn_mean", name="bn_mean")
                nc.vector.tensor_scalar_mul(mean[:], ssum, 1.0 / count)
                ex2 = wk.tile([128, 1], F32, tag="bn_ex2", name="bn_ex2")
                nc.vector.tensor_scalar_mul(ex2[:], ssq, 1.0 / count)
                m2 = wk.tile([128, 1], F32, tag="bn_m2", name="bn_m2")
                nc.vector.tensor_mul(m2[:], mean[:], mean[:])
                var = wk.tile([128, 1], F32, tag="bn_var", name="bn_var")
                nc.vector.tensor_sub(var[:], ex2[:], m2[:])
                nc.vector.tensor_scalar_add(var[:], var[:], BN_EPS)
                inv = wk.tile([128, 1], F32, tag="bn_inv", name="bn_inv")
                nc.vector.reciprocal(inv[:], var[:])
                rs = wk.tile([128, 1], F32, tag="bn_rs", name="bn_rs")
                nc.scalar.sqrt(rs[:], inv[:])
                k = wk.tile([128, 1], F32, tag="bn_k", name="bn_k")
                nc.vector.tensor_mul(k[:], rs[:], gamma_beta[:, 0:1])
                mk = wk.tile([128, 1], F32, tag="bn_mk", name="bn_mk")
                nc.vector.tensor_mul(mk[:], mean[:], k[:])
                b = wk.tile([128, 1], F32, tag="bn_b", name="bn_b")
                nc.vector.tensor_sub(b[:], gamma_beta[:, 1:2], mk[:])
                return k, b

            def stats_of(ytile, width):
                """Local per-feature sum and sum-of-squares of y [128,width]."""
                ssum = wk.tile([128, 1], F32, tag="st_ssum", name="st_ssum")
                nc.vector.tensor_reduce(
                    ssum[:], ytile[:, 0:width], axis=mybir.AxisListType.X,
                    op=mybir.AluOpType.add,
                )
                sq = wk.tile([128, len(dts)], F32, tag="st_sq", name="st_sq")
                for i, (o0, w_) in enumerate(dts):
                    scr = wk.tile([128, fts], BF16, tag="scr0", name="scr0")
                    nc.scalar.activation(
                        scr[:, 0:w_], ytile[:, o0 : o0 + w_],
                        mybir.ActivationFunctionType.Square,
                        accum_out=sq[:, i : i + 1],
                    )
                ssq = wk.tile([128, 1], F32, tag="st_ssq", name="st_ssq")
                nc.vector.tensor_reduce(
                    ssq[:], sq[:], axis=mybir.AxisListType.X,
                    op=mybir.AluOpType.add
                )
                return ssum, ssq

            def allreduce_stats(ssum, ssq, idx):
                statloc = wk.tile([128, 2], F32, tag="statloc", name="statloc")
                nc.vector.tensor_copy(statloc[:, 0:1], ssum[:])
                nc.vector.tensor_copy(statloc[:, 1:2], ssq[:])
                nc.sync.dma_start(cc_stat_in[:, :], statloc[:])
                nc.gpsimd.collective_compute(
                    "AllReduce", mybir.AluOpType.add, groups,
                    [cc_stat_in[:, :].opt()], [cc_stat_out[idx][:, :].opt()],
                )
                statglob = wk.tile([128, 2], F32, tag="statglob",
                                   name="statglob")
                nc.sync.dma_start(statglob[:], cc_stat_out[idx][:, :])
                return statglob

            GW = 8  # node tiles per batched row-write DMA

            def write_rows(cols, dst_dram, width, src_dt, grp=GW,
                           fill_pad=False):
                """Transpose f-major own tiles into row-major dst_dram.

                cols: list of (col_off, ncol, get_ap(t, nt) -> [ncol, nt] AP).
                """
                full = own // 128
                tail = own % 128
                ident_t = id_bf if src_dt == BF16 else id_f32

                def rowbuf_tile():
                    rb = wk.tile([128, grp * width], src_dt,
                                 tag=f"rows{width}_{src_dt}",
                                 name=f"rows{width}_{src_dt}")
                    if fill_pad:
                        nc.vector.memset(rb[:], 0.0)
                    return rb

                def do_tile(rowbuf, t, nt, g):
                    for (co, ncol, get) in cols:
                        src = get(t, nt)
                        pt = pst.tile([128, 128], src_dt, tag="t", name="pt")
                        nc.tensor.transpose(
                            pt[0:nt, 0:ncol], src, ident_t[0:ncol, 0:ncol]
                        )
                        nc.scalar.copy(
                            rowbuf[0:nt, g * width + co : g * width + co + ncol],
                            pt[0:nt, 0:ncol],
                        )

                t = 0
                while t < full:
                    gcnt = min(grp, full - t)
                    rowbuf = rowbuf_tile()
                    for g in range(gcnt):
                        do_tile(rowbuf, t + g, 128, g)
                    if gcnt == 1:
                        nc.sync.dma_start(
                            dst_dram[t * 128 : (t + 1) * 128, :],
                            rowbuf[:, 0:width],
                        )
                    else:
                        dst = dst_dram[t * 128 : (t + gcnt) * 128, :].rearrange(
                            "(g p) f -> p g f", p=128
                        )
                        nc.sync.dma_start(
                            dst,
                            rowbuf[:, 0 : gcnt * width].rearrange(
                                "p (g f) -> p g f", f=width
                            ),
                        )
                    t += gcnt
                if tail:
                    rowbuf = rowbuf_tile()
                    do_tile(rowbuf, full, tail, 0)
                    nc.sync.dma_start(
                        dst_dram[full * 128 : own, :], rowbuf[0:tail, 0:width]
                    )

            def dense_own(lhsTs, rhs_fn, out_tag, out_dt, pool, bias=None,
                          act=None, scale=None):
                """out[128, own] = act(scale * sum_i lhsTs[i].T @ rhs_i + bias)."""
                o = pool.tile([128, own], out_dt, tag=out_tag, name=out_tag)
                if act is not None:
                    func = act
                elif bias is None and scale is None:
                    func = mybir.ActivationFunctionType.Copy
                else:
                    func = mybir.ActivationFunctionType.Identity
                for (o0, w_) in dts:
                    pt = psd.tile([128, 512], F32, tag="d", name="pd")
                    for i, (lt, rf) in enumerate(zip(lhsTs, rhs_fn)):
                        nc.tensor.matmul(
                            pt[:, 0:w_], lt, rf(o0, w_),
                            start=(i == 0), stop=(i == len(lhsTs) - 1),
                        )
                    kw = {}
                    if bias is not None:
                        kw["bias"] = bias
                    if scale is not None:
                        kw["scale"] = scale
                    nc.scalar.activation(
                        o[:, o0 : o0 + w_], pt[:, 0:w_], func, **kw,
                    )
                return o

            qrr = [0]  # SWDGE queue round-robin: spread desc-gen over Q7 pairs
            cpb2 = cpb // 2

            def gather_block(pool, table, idx16, b, width, tag, bufs=3):
                """Gather one block's spb rows into [128,cpb,width], split into
                two half-block dma_gathers on different SWDGE queues so the
                Q7 descriptor generation runs on two core pairs in parallel."""
                t = pool.tile([128, cpb, width], BF16, tag=tag, name=tag,
                              bufs=bufs)
                for (c0, c1) in ((0, cpb2), (cpb2, cpb)):
                    nchunk = c1 - c0
                    nc.gpsimd.dma_gather(
                        t[:, c0:c1, :], table[:, :],
                        idx16[:, b * ipb + c0 * 8 : b * ipb + c1 * 8],
                        nchunk * 128, nchunk * 128, width,
                        single_packet=False, queue_num=qrr[0],
                    )
                    qrr[0] = (qrr[0] + 1) % 4
                return t

            def sel_block(pool, b, tag, bufs=2):
                """sel[e, c, d] = (iota_d == dstw[e, c]) for one block."""
                s = pool.tile([128, cpb, 128], BF16, tag=tag, name=tag,
                              bufs=bufs)
                io_b = _insert_bcast(iota[:, :], 1, cpb)
                dw_b = _insert_bcast(dstw[:, b * cpb : (b + 1) * cpb], 2, 128)
                nc.vector.tensor_tensor(s[:, :, :], io_b, dw_b,
                                        op=mybir.AluOpType.is_equal)
                return s

            # =================================================================
            # Phase 0: embedding (own slice) + stats AllReduce + x0 AllGather
            # =================================================================
            with tc.tile_pool(name="p0", bufs=1) as p0:
                nfT = load("nfT", [din, own], BF16, pool=p0)
                y0 = p0.tile([128, own], BF16, tag="y0", name="y0")
                for (o0, w_) in dts:
                    pt = psd.tile([128, 512], F32, tag="d", name="pd")
                    nc.tensor.matmul(
                        pt[:, 0:w_], w_emb[:], nfT[:, o0 : o0 + w_],
                        start=True, stop=True,
                    )
                    nc.scalar.copy(y0[:, o0 : o0 + w_], pt[:, 0:w_])
                ssum0, ssq0 = stats_of(y0, own)
                statglob = allreduce_stats(ssum0, ssq0, 2)
                k0, b0 = bn_cols_from_stats(
                    statglob[:, 0:1], statglob[:, 1:2], bn_emb, n
                )
                x0 = p0.tile([128, own], BF16, tag="x0", name="x0")
                nc.scalar.activation(
                    x0[:], y0[:], mybir.ActivationFunctionType.Relu,
                    bias=b0[:], scale=k0[:],
                )
                write_rows(
                    [(0, C, lambda t, nt: x0[:, t * 128 : t * 128 + nt])],
                    cc_rows_x, C, BF16,
                )
                nc.gpsimd.collective_compute(
                    "AllGather", mybir.AluOpType.bypass, groups,
                    [cc_rows_x[:, :].opt()], [x_table[0][:, :].opt()],
                )

            # =================================================================
            # GIN layers (x2): edge aggregation + own-only dense MLP
            # =================================================================
            def gin_layer(li, sp, out_pool):
                h = sp.tile([128, own], BF16, tag=f"h_gin{li}",
                            name=f"h_gin{li}")
                with (
                    tc.tile_pool(name=f"gin_e{li}", bufs=2) as ep,
                    tc.tile_pool(name=f"gin_p{li}", bufs=2, space="PSUM") as pp,
                ):
                    for b in range(nblk):
                        gt = gather_block(ep, x_table[li], idx_src, b, C, "gt")
                        s = sel_block(ep, b, "sel")
                        pa = pp.tile([128, 128], F32, tag="agg", name="agg")
                        for c in range(cpb):
                            nc.tensor.matmul(
                                pa[:], gt[:, c, :], s[:, c, :],
                                start=(c == 0), stop=(c == cpb - 1),
                            )
                        nb = ntile_own[b]
                        nc.scalar.copy(
                            h[:, b * 128 : b * 128 + nb], pa[:, 0:nb]
                        )
                # dense: y = h @ W1; BN(global) + relu; x = h2 @ W2 + b2
                y = dense_own([w1[:]], [lambda o0, w_: h[:, o0 : o0 + w_]],
                              f"y_gin{li}", BF16, sp)
                ssum, ssq = stats_of(y, own)
                statglob = allreduce_stats(ssum, ssq, li)
                k, bcol = bn_cols_from_stats(
                    statglob[:, 0:1], statglob[:, 1:2], bn_gin, n
                )
                h2 = sp.tile([128, own], BF16, tag=f"h2_gin{li}",
                             name=f"h2_gin{li}")
                nc.scalar.activation(
                    h2[:], y[:], mybir.ActivationFunctionType.Relu,
                    bias=bcol[:], scale=k[:],
                )
                x = dense_own(
                    [w2[:]], [lambda o0, w_: h2[:, o0 : o0 + w_]],
                    f"x_gin{li}", BF16, out_pool,
                    bias=b2c[:], act=mybir.ActivationFunctionType.Identity,
                )
                if li == 0:
                    write_rows(
                        [(0, C, lambda t, nt: x[:, t * 128 : t * 128 + nt])],
                        cc_rows_x, C, BF16,
                    )
                    nc.gpsimd.collective_compute(
                        "AllGather", mybir.AluOpType.bypass, groups,
                        [cc_rows_x[:, :].opt()], [x_table[1][:, :].opt()],
                    )
                return x

            if phase_limit >= 1:
                with tc.tile_pool(name="g0", bufs=1) as g0p:
                    gin_layer(0, g0p, g0p)

            zb = st.tile([128, own], BF16, tag="zb", name="zb")
            with tc.tile_pool(name="g1", bufs=1) as g1p:
                if phase_limit < 2:
                    nc.vector.memset(zb[:], 0.0)
                    x2 = None
                else:
                    x2 = gin_layer(1, g1p, g1p)

                # =============================================================
                # VAE heads (own only)
                # =============================================================
                if phase_limit < 3:
                    zt = wk.tile([128, 512], F32, tag="zf", name="zf")
                    nc.vector.memset(zt[:], 0.0)
                    for nm in ("mu_s", "logvar_s", "zin_s"):
                        for t in range(nblk):
                            nt = ntile_own[t]
                            nc.sync.dma_start(
                                outs[nm][t * 128 : t * 128 + nt, :],
                                zt[0:nt, 0:C])
                    mu = None
                else:
                    mu = dense_own(
                        [w_mu[:]], [lambda o0, w_: x2[:, o0 : o0 + w_]],
                        "mu", F32, g1p, bias=bmuc[:],
                        act=mybir.ActivationFunctionType.Identity)
                if phase_limit >= 3:
                    lv = dense_own(
                        [w_var[:]], [lambda o0, w_: x2[:, o0 : o0 + w_]],
                        "lv", F32, g1p, bias=bvarc[:],
                        act=mybir.ActivationFunctionType.Identity)
                    eh = g1p.tile([128, own], F32, tag="eh", name="eh")
                    nc.scalar.activation(
                        eh[:], lv[:], mybir.ActivationFunctionType.Exp,
                        scale=0.5
                    )
                    epsT = load("epsT", [128, own], F32, pool=g1p)
                    z = g1p.tile([128, own], F32, tag="z", name="z")
                    nc.vector.tensor_mul(z[:], epsT[:], eh[:])
                    nc.vector.tensor_add(z[:], z[:], mu[:])
                    nc.vector.tensor_copy(zb[:], z[:])
                    write_rows(
                        [(0, C, lambda t, nt: mu[:, t * 128 : t * 128 + nt])],
                        outs["mu_s"], C, F32)
                    write_rows(
                        [(0, C, lambda t, nt: lv[:, t * 128 : t * 128 + nt])],
                        outs["logvar_s"], C, F32)
                    write_rows(
                        [(0, C, lambda t, nt: z[:, t * 128 : t * 128 + nt])],
                        outs["zin_s"], C, F32)

            # =================================================================
            # GAT layers (x2)
            # =================================================================
            def gat_tables(li, act):
                """act [128, own] bf16 -> big_table[li] (AllGather), ad_pad[li]
                (local)."""
                with tc.tile_pool(name=f"gtab{li}", bufs=1) as tp:
                    hh = [
                        dense_own(
                            [w_gat[:, hd * C : (hd + 1) * C]],
                            [lambda o0, w_: act[:, o0 : o0 + w_]],
                            f"hh{hd}_l{li}", BF16, tp,
                        )
                        for hd in range(H)
                    ]
                    # a[kind][hd]: [1, own] row, kind 0 = a_src, 1 = a_dst
                    arow = [[None, None], [None, None]]
                    for hd in range(H):
                        for kind in range(2):
                            t_ = tp.tile([1, own], BF16,
                                         tag=f"a{kind}{hd}_l{li}",
                                         name=f"a{kind}{hd}_l{li}")
                            for (o0, w_) in dts:
                                pt = psd.tile([1, 512], F32, tag="d", name="pd")
                                nc.tensor.matmul(
                                    pt[0:1, 0:w_],
                                    attp[hd][:, kind : kind + 1],
                                    hh[hd][:, o0 : o0 + w_],
                                    start=True, stop=True,
                                )
                                nc.scalar.copy(t_[0:1, o0 : o0 + w_],
                                               pt[0:1, 0:w_])
                            arow[kind][hd] = t_
                    write_rows(
                        [
                            (0, C,
                             lambda t, nt: hh[0][:, t * 128 : t * 128 + nt]),
                            (C, C,
                             lambda t, nt: hh[1][:, t * 128 : t * 128 + nt]),
                            (2 * C, 1,
                             lambda t, nt: arow[0][0][0:1,
                                                      t * 128 : t * 128 + nt]),
                            (2 * C + 1, 1,
                             lambda t, nt: arow[0][1][0:1,
                                                      t * 128 : t * 128 + nt]),
                        ],
                        cc_rows_big, bigw, BF16, fill_pad=True,
                    )
                    write_rows(
                        [
                            (0, 1,
                             lambda t, nt: arow[1][0][0:1,
                                                      t * 128 : t * 128 + nt]),
                            (1, 1,
                             lambda t, nt: arow[1][1][0:1,
                                                      t * 128 : t * 128 + nt]),
                        ],
                        ad_pad[li], 128, BF16, fill_pad=True,
                    )
                nc.gpsimd.collective_compute(
                    "AllGather", mybir.AluOpType.bypass, groups,
                    [cc_rows_big[:, :].opt()], [big_table[li][:, :].opt()],
                )

            def gat_edge_dec(li, out_pool, out_dt):
                """Edge softmax + message aggregation + decoder matmul.

                Per chunk: psum[dst, 0:258] += sel.T @ [el0*hh0|el1*hh1|el0,el1]
                so the denominator comes out of the same matmul; division is a
                per-partition scale at evacuation, then PE transpose to
                feature-major.
                """
                with (
                    tc.tile_pool(name=f"gat_s{li}", bufs=1) as gsp,
                    tc.tile_pool(name=f"gat_e{li}", bufs=2) as ep,
                    tc.tile_pool(name=f"gat_p{li}", bufs=2, space="PSUM") as pp,
                ):
                    on = [gsp.tile([128, own], BF16, tag=f"on{hd}",
                                   name=f"on{hd}")
                          for hd in range(H)]
                    for b in range(nblk):
                        gt = gather_block(ep, big_table[li], idx_src, b, bigw,
                                          "gt")
                        adg = gather_block(ep, ad_pad[li], idx_dstl, b, 128,
                                           "adg", bufs=2)
                        s = sel_block(ep, b, "sel")
                        # el = exp(leaky(a_s + a_d)) -> gt[:, :, 256:258]
                        lt = ep.tile([128, cpb, 2], F32, tag="lt", name="lt")
                        nc.vector.tensor_add(
                            lt[:, :, :], gt[:, :, 2 * C : 2 * C + 2],
                            adg[:, :, 0:2],
                        )
                        nc.vector.scalar_tensor_tensor(
                            lt[:, :, :], lt[:, :, :], 0.2, lt[:, :, :],
                            mybir.AluOpType.mult, mybir.AluOpType.max,
                        )
                        nc.scalar.activation(
                            gt[:, :, 2 * C : 2 * C + 2], lt[:, :, :],
                            mybir.ActivationFunctionType.Exp,
                        )
                        # scale hh halves by el in place
                        for hd in range(H):
                            el_b = gt[:, :, 2 * C + hd : 2 * C + hd + 1]
                            el_b = AP(el_b.tensor, el_b.offset,
                                      [list(el_b.ap[0]), list(el_b.ap[1]),
                                       [0, 128]])
                            nc.vector.tensor_mul(
                                gt[:, :, hd * C : (hd + 1) * C],
                                gt[:, :, hd * C : (hd + 1) * C],
                                el_b,
                            )
                        pa = pp.tile([128, 258], F32, tag="pa", name="pa")
                        for c in range(cpb):
                            nc.tensor.matmul(
                                pa[:, 0:258], s[:, c, :],
                                gt[:, c, 0 : 2 * C + 2],
                                start=(c == 0), stop=(c == cpb - 1),
                            )
                        nb = ntile_own[b]
                        r2 = wk.tile([128, 2], F32, tag="r2", name="r2")
                        nc.vector.reciprocal(r2[0:nb, :],
                                             pa[0:nb, 2 * C : 2 * C + 2])
                        for hd in range(H):
                            sb = wk.tile([128, 128], BF16, tag="sb", name="sb")
                            nc.scalar.activation(
                                sb[0:nb, :],
                                pa[0:nb, hd * C : (hd + 1) * C],
                                mybir.ActivationFunctionType.Identity,
                                scale=r2[0:nb, hd : hd + 1],
                            )
                            ptr = pst.tile([128, 128], BF16, tag="t",
                                           name="ptr")
                            nc.tensor.transpose(
                                ptr[0:128, 0:nb], sb[0:nb, 0:128],
                                id_bf[0:nb, 0:nb],
                            )
                            nc.scalar.copy(
                                on[hd][:, b * 128 : b * 128 + nb],
                                ptr[:, 0:nb],
                            )
                    # decoder: zout = (out + b_gat) @ W_dec + b_dec (prefolded)
                    zo = dense_own(
                        [w_dec0[:], w_dec1[:]],
                        [lambda o0, w_: on[0][:, o0 : o0 + w_],
                         lambda o0, w_: on[1][:, o0 : o0 + w_]],
                        f"zo_l{li}", out_dt, out_pool,
                        bias=bdecc[:], act=mybir.ActivationFunctionType.Identity,
                    )
                return zo

            if phase_limit >= 4:
                gat_tables(0, zb)
            if phase_limit >= 5:
                zo1 = gat_edge_dec(0, st, BF16)
            if phase_limit >= 6:
                gat_tables(1, zo1)
            if phase_limit >= 7:
                with tc.tile_pool(name="zo2p", bufs=1) as zp:
                    zo2 = gat_edge_dec(1, zp, F32)
                    write_rows(
                        [(0, C, lambda t, nt: zo2[:, t * 128 : t * 128 + nt])],
                        outs["zout_s"], C, F32,
                    )
            else:
                zt0 = wk.tile([128, 512], F32, tag="zf", name="zf")
                nc.vector.memset(zt0[:], 0.0)
                for t in range(nblk):
                    nt = ntile_own[t]
                    nc.sync.dma_start(
                        outs["zout_s"][t * 128 : t * 128 + nt, :],
                        zt0[0:nt, 0:C])
    nc.compile()
    return nc


# =====================================================================
# Host side
# =====================================================================
def host_prep(edge_index, cfg: Cfg):
    n, ncores, own, nblk = cfg.n, cfg.ncores, cfg.own, cfg.nblk
    src = np.asarray(edge_index[0], dtype=np.int64)
    dst = np.asarray(edge_index[1], dtype=np.int64)
    loop = np.arange(n, dtype=np.int64)
    src = np.concatenate([src, loop])
    dst = np.concatenate([dst, loop])
    order = np.argsort(dst, kind="stable")
    src, dst = src[order], dst[order]
    # bucket edges by (core, block)
    core_of = dst // own
    blk_in_core = (dst - core_of * own) // 128
    counts = np.zeros((ncores, nblk), dtype=np.int64)
    np.add.at(counts, (core_of, blk_in_core), 1)
    cpb = int(-(-counts.max() // 128))
    cfg.cpb = cpb
    slots = cfg.slots
    idx_src = np.zeros((ncores, slots), dtype=np.int64)
    idx_dstl = np.zeros((ncores, slots), dtype=np.int64)
    dstl = np.full((ncores, slots), -1.0, dtype=np.float32)
    # edges are sorted by dst, so per (core, block) they are contiguous
    for c in range(ncores):
        for b in range(nblk):
            b_lo = np.searchsorted(dst, c * own + b * 128)
            b_hi = np.searchsorted(dst,
                                   min(c * own + (b + 1) * 128, (c + 1) * own))
            cnt = b_hi - b_lo
            s0 = b * cpb * 128
            idx_src[c, s0 : s0 + cnt] = src[b_lo:b_hi]
            idx_dstl[c, s0 : s0 + cnt] = dst[b_lo:b_hi] - c * own
            dstl[c, s0 : s0 + cnt] = (
                dst[b_lo:b_hi] - c * own - b * 128
            ).astype(np.float32)

    def wrap16(a):
        # slot j of each block -> [j%16 + 16k, j//16] within the block cols
        blocks = a.reshape(nblk, cfg.spb)
        w = np.stack([np.tile(blk.reshape(cfg.ipb, 16).T, (8, 1))
                      for blk in blocks], axis=0)  # [nblk, 128, ipb]
        return np.ascontiguousarray(
            w.transpose(1, 0, 2).reshape(128, nblk * cfg.ipb)
        ).astype(np.int16)

    per_core = {}
    for c in range(ncores):
        per_core[c] = dict(
            idx_src=wrap16(idx_src[c]),
            idx_dstl=wrap16(idx_dstl[c]),
            dstw=np.ascontiguousarray(
                dstl[c].reshape(slots // 128, 128).T
            ),
        )
    return per_core


def kernel(node_features_s, edge_index_s, eps_noise,
           W_emb, b_emb, g_emb, be_emb,
           W1, b1, g1, be1, W2, b2,
           W_mu, b_mu, W_var, b_var,
           W_gat, att_src, att_dst, b_gat,
           W_dec, b_dec, _cfg=None, _nc_cache={}):
    cfg = _cfg or Cfg()
    n, own, C, H = cfg.n, cfg.own, cfg.c, cfg.h
    per_core = host_prep(edge_index_s, cfg)

    nf = np.asarray(node_features_s, dtype=np.float32)
    eps = np.asarray(eps_noise, dtype=np.float32)

    def colpair(a, b_):
        return np.stack([np.asarray(a, np.float32).reshape(-1),
                         np.asarray(b_, np.float32).reshape(-1)], axis=1)

    iota = np.tile(np.arange(128, dtype=np.float32), (128, 1))
    bdec_eff = (np.asarray(b_gat, np.float32) @ np.asarray(W_dec, np.float32)
                + np.asarray(b_dec, np.float32))
    shared = dict(
        w_emb=np.asarray(W_emb, np.float32).astype(NPBF),
        w1=np.asarray(W1, np.float32).astype(NPBF),
        w2=np.asarray(W2, np.float32).astype(NPBF),
        w_mu=np.asarray(W_mu, np.float32).astype(NPBF),
        w_var=np.asarray(W_var, np.float32).astype(NPBF),
        w_gat=np.asarray(W_gat, np.float32).astype(NPBF),
        w_dec0=np.asarray(W_dec, np.float32)[:C].astype(NPBF),
        w_dec1=np.asarray(W_dec, np.float32)[C:].astype(NPBF),
        attp0=np.stack([np.asarray(att_src, np.float32)[0],
                        np.asarray(att_dst, np.float32)[0]],
                       axis=1).astype(NPBF),
        attp1=np.stack([np.asarray(att_src, np.float32)[1],
                        np.asarray(att_dst, np.float32)[1]],
                       axis=1).astype(NPBF),
        bn_emb=colpair(g_emb, be_emb),
        bn_gin=colpair(g1, be1),
        b2c=np.asarray(b2, np.float32).reshape(C, 1),
        bmuc=np.asarray(b_mu, np.float32).reshape(C, 1),
        bvarc=np.asarray(b_var, np.float32).reshape(C, 1),
        bdecc=bdec_eff.reshape(C, 1),
        iota=iota.astype(NPBF),
        id_bf=np.eye(128, dtype=np.float32).astype(NPBF),
        id_f32=np.eye(128, dtype=np.float32),
    )
    in_maps = []
    for c in range(cfg.ncores):
        m = dict(shared)
        m["nfT"] = np.ascontiguousarray(
            nf[c * own : (c + 1) * own].T).astype(NPBF)
        m["epsT"] = np.ascontiguousarray(eps[c * own : (c + 1) * own].T)
        m.update(per_core[c])
        in_maps.append(m)

    pl = int(os.environ.get("KERNEL_PHASES", "99"))
    key = (cfg.n, cfg.ncores, cfg.cpb, pl, SHARED)
    if key not in _nc_cache:
        _nc_cache[key] = build_program(cfg, phase_limit=pl)
    nc = _nc_cache[key]

    res = run_bass_kernel_spmd(
        nc, in_maps, core_ids=list(range(cfg.ncores)),
        trace=bool(int(os.environ.get("KERNEL_TRACE", "0"))),
    )
    results = res.results
    kernel.last_run = res

    def stitch(name):
        return np.concatenate([np.asarray(results[c][name], np.float32)
                               for c in range(cfg.ncores)], axis=0)

    return (stitch("zin_s"), stitch("zout_s"), stitch("mu_s"),
            stitch("logvar_s"))
